# revision 15
# baseline (speedup 1.0000x reference)
"""Trainium2 Bass kernel for nn_FIN_b: windowed-FM tabular net.

Data-parallel over batch: B=2048 rows split across 8 NeuronCores (256 each).
Activations are feature-major ([feature_partition, batch_free]).  Front and
big matmuls run in bf16, the FM block in fp8-e4m3 (both fine vs the required
tolerance), fp32 PSUM accumulate everywhere.  The windowed FM
    fm_out[b,c] = sum_{d=1..7} sum_f D_d[b,c+f] G[c,f,f+d],
    D_d = x * shift_d(x),  G[c,f,g] = sum_e v[c,f,e] v[c,g,e]
runs in nine 121-channel blocks: x is written (fp8) to a feature-major DRAM
scratch as each front activation finishes; for block Cb ONE linear DMA
fetches rows 121*Cb + p + d (p=0..127, d=0..7) giving the block's x window
plus all 7 shifted windows (121+7 == 128, so no boundary stragglers), one
broadcast multiply forms the 7 D_d products, 7 banded matmuls accumulate fm,
and the block's 8 W1b contributions to the big matmul are issued right away.
The big-matmul x-half streams afterwards, interleaved with activations and
the W2 dot per output tile.  The FM linear term (x_fm @ lin_w) is folded
into W1's top half on the host.

Scheduling notes (why this is fast):
 - The PE p-state ramps to full clock only under continuous load, so the
   emission order keeps one long tensor stream: front (kt-outer) -> per-block
   FM+W1b -> W1a+W2.
 - The two HW DGE queues are split so small data-path packets never fight
   bulk weight packets for DMA-engine slots: Activation queue carries
   bias/Wf/Gm then the nine shifted reads; Sync queue carries xin, the x
   writes, then W1b (nine chunks) / W1a / out strictly FIFO.
"""

import sys

sys.path.insert(0, "/opt/trn_rl_repo")

import numpy as np
import ml_dtypes

import concourse.bass as bass
import concourse.tile as tile
from concourse import bacc, mybir
from concourse.bass_utils import run_bass_kernel_spmd

NDF, NCF, NCC = 512, 256, 256
EMB, FIELD = 16, 8
B = 2048
NH0 = NDF + 2 * NCC          # 1024
CHANNEL = NH0 - FIELD + 1    # 1017
HID = (NH0 + CHANNEL) // 2   # 1020
NCORES = 8
BC = B // NCORES             # 256 batch rows per core
CB = 121                     # channels per FM block (121 + 7 shifts = 128)
NCB = 9                      # ceil(CHANNEL / CB)
XPAD = CB * (NCB - 1) + 128 + FIELD   # pad rows so every block read is legal

F32 = mybir.dt.float32
BF16 = mybir.dt.bfloat16
FP8 = mybir.dt.bfloat16  # FM path dtype (DVE fp8 is 1.8x slower)

_cache = {}


def _build(b2_val: float):
    nc = bacc.Bacc()

    xin_d = nc.dram_tensor("xin", [128, 6, BC], BF16, kind="ExternalInput")
    Wf_d = nc.dram_tensor("Wf", [128, 6, 512], BF16, kind="ExternalInput")
    Gm_d = nc.dram_tensor("Gm", [128, 7, NCB, CB], FP8, kind="ExternalInput")
    W1a_d = nc.dram_tensor("W1a", [128, 8, 1024], BF16, kind="ExternalInput")
    W1b_d = nc.dram_tensor("W1b", [128, NCB, 1024], BF16, kind="ExternalInput")
    bias_d = nc.dram_tensor("bias", [128, 16], F32, kind="ExternalInput")
    W2_d = nc.dram_tensor("W2", [128, 8, 1], BF16, kind="ExternalInput")
    out_d = nc.dram_tensor("out", [1, BC], F32, kind="ExternalOutput")

    AF = mybir.ActivationFunctionType

    with tile.TileContext(nc) as tc:
        xpad, _xpad_free = tc.tile([XPAD, BC], FP8, space="DRAM", name="xpad")
        with (
            tc.tile_pool(name="w", bufs=1) as wp,
            tc.tile_pool(name="act", bufs=1) as ap,
            tc.tile_pool(name="xa", bufs=3) as xap,
            tc.tile_pool(name="dt", bufs=2) as dtp,
            tc.tile_pool(name="pfr", bufs=2, space=bass.MemorySpace.PSUM) as pfr,
            tc.tile_pool(name="pm1", bufs=1, space=bass.MemorySpace.PSUM) as pm1,
            tc.tile_pool(name="pfm", bufs=2, space=bass.MemorySpace.PSUM) as pfm,
        ):
            # ---- Activation-queue DMAs: bias/Wf/Gm (the shifted xa reads
            # ---- are issued later from the same engine) ----
            bias = wp.tile([128, 16], F32, tag="bias")
            nc.scalar.dma_start(bias[:], bias_d[:])
            Wf = wp.tile([128, 6, 512], BF16, tag="Wf")
            nc.scalar.dma_start(Wf[:, 0, :], Wf_d[:, 0, :])
            nc.scalar.dma_start(Wf[:, 1, :], Wf_d[:, 1, :])
            nc.scalar.dma_start(Wf[:, 2:4, :], Wf_d[:, 2:4, :])
            nc.scalar.dma_start(Wf[:, 4:6, :], Wf_d[:, 4:6, :])
            Gm = wp.tile([128, 7, NCB, CB], FP8, tag="Gm")
            nc.scalar.dma_start(Gm[:], Gm_d[:])
            W2 = wp.tile([128, 8, 1], BF16, tag="W2")
            nc.scalar.dma_start(W2[:], W2_d[:])

            # ---- Sync-queue: input now; x writes / W1b / W1a follow FIFO ----
            xin = wp.tile([128, 6, BC], BF16, tag="xin")
            nc.sync.dma_start(xin[:], xin_d[:])

            zst = ap.tile([128, 128], BF16, tag="zst")
            nc.vector.memset(zst[:], 0.0)
            zer = ap.tile([XPAD - NH0, BC], FP8, tag="zer")
            nc.vector.memset(zer[:], 0.0)
            nc.gpsimd.dma_start(xpad[NH0:XPAD, :], zer[:])

            x = ap.tile([128, 8, BC], BF16, tag="x")
            fmbf = ap.tile([128, NCB, BC], BF16, tag="fmbf")
            nc.vector.memset(fmbf[:], 0.0)
            h = ap.tile([128, 8, BC], BF16, tag="h")
            sig = ap.tile([1, BC], F32, tag="sig")

            # ---- front, kt-outer so matmuls start on the first Wf chunk ----
            fps = [pfr.tile([128, 2 * BC], F32, tag="pfr", name=f"fps{j}")
                   for j in range(2)]
            facc = lambda mt: fps[mt // 2][:, (mt % 2) * BC:(mt % 2 + 1) * BC]
            # a matmul with start=True clears its whole PSUM bank, so paired
            # accumulator banks are zeroed once with a zero-stationary matmul
            # and every real chain accumulates with start=False
            for j in range(2):
                nc.tensor.matmul(fps[j][:], zst[:], Wf[:, 0, :],
                                 start=True, stop=False)
            for kt in range(4):
                for mt in range(4):
                    nc.tensor.matmul(
                        facc(mt), Wf[:, kt, mt * 128:(mt + 1) * 128],
                        xin[:, kt, :], start=False, stop=(kt == 3),
                    )
            # d-part activations first: they free the psum pair banks the
            # c-part accumulators will rotate into (WAR must see the readers)
            for mt in range(4):
                nc.scalar.activation(
                    out=x[:, mt, :], in_=facc(mt), func=AF.Lrelu,
                    bias=bias[:, mt:mt + 1], scale=1.0, alpha=0.0,
                )
                if mt % 2 == 1:
                    nc.gpsimd.dma_start(
                        xpad[(mt - 1) * 128:(mt + 1) * 128, :]
                        .rearrange("(g p) b -> p g b", p=128),
                        x[:, mt - 1:mt + 1, :])
            cps = [pfr.tile([128, 2 * BC], F32, tag="pfr", name=f"cps{j}")
                   for j in range(2)]
            cacc = lambda mt: cps[mt // 2][:, (mt % 2) * BC:(mt % 2 + 1) * BC]
            for j in range(2):
                nc.tensor.matmul(cps[j][:], zst[:], Wf[:, 0, :],
                                 start=True, stop=False)
            for kt in range(2):
                for mt in range(4):
                    nc.tensor.matmul(
                        cacc(mt), Wf[:, 4 + kt, mt * 128:(mt + 1) * 128],
                        xin[:, 4 + kt, :], start=False, stop=(kt == 1),
                    )
            # xa_0 issued here so it only depends on the d-part writes
            xas = []
            def xa_read(Cb):
                xa = xap.tile([128, 8, BC], FP8, tag="xa", name="xa")
                # xa[p, d, :] = xpad[121*Cb + p + d, :]  (one linear DMA)
                src = bass.AP(xpad.tensor, CB * Cb * BC,
                              [[BC, 128], [BC, 8], [1, BC]])
                nc.scalar.dma_start(xa[:], src)
                return xa
            xas.append(xa_read(0))
            for mt in range(4):
                nc.scalar.activation(
                    out=x[:, 4 + mt, :], in_=cacc(mt), func=AF.Lrelu,
                    bias=bias[:, 4 + mt:5 + mt], scale=1.0, alpha=0.0,
                )
            nc.gpsimd.dma_start(
                xpad[512:1024, :].rearrange("(g p) b -> p g b", p=128),
                x[:, 4:8, :])

            # ---- mm1 accumulators + weight streams (W1a first: its chunks
            # ---- fill the front->FM bubble; W1b chunks arrive per-block) ----
            m1ps = [pm1.tile([128, 2 * BC], F32, tag=f"pm1{j}", name=f"pm1{j}")
                    for j in range(4)]
            m1acc = lambda mt: m1ps[mt // 2][:, (mt % 2) * BC:(mt % 2 + 1) * BC]
            for j in range(4):
                nc.tensor.matmul(m1ps[j][:], zst[:], Wf[:, 0, :],
                                 start=True, stop=False)

            W1a = wp.tile([128, 8, 1024], BF16, tag="W1a")
            for kt in range(8):
                nc.sync.dma_start(W1a[:, kt, :], W1a_d[:, kt, :])
            W1b = wp.tile([128, NCB, 1024], BF16, tag="W1b")
            for j in range(3):
                nc.sync.dma_start(W1b[:, 3 * j:3 * j + 3, :],
                                  W1b_d[:, 3 * j:3 * j + 3, :])

            # x-half of the big matmul, kt 0..3: fills the tensor bubble while
            # the first shifted read + D product are in flight
            for kt in range(4):
                for mt in range(8):
                    nc.tensor.matmul(
                        m1acc(mt), W1a[:, kt, mt * 128:(mt + 1) * 128],
                        x[:, kt, :], start=False, stop=False,
                    )

            # ---- FM + W1b pipeline over nine 121-channel blocks ----
            for Cb in range(NCB):
                xa = xas[Cb]
                if Cb + 1 < NCB:
                    xas.append(xa_read(Cb + 1))
                Dt = dtp.tile([128, 7, BC], FP8, tag="Dt")
                nc.vector.tensor_mul(
                    Dt[:], xa[:, 0:1, :].broadcast_to([128, 7, BC]),
                    xa[:, 1:8, :])
                fmp = pfm.tile([121, BC], F32, tag="pfm", name="fmp")
                for d in range(1, 8):
                    nc.tensor.matmul(
                        fmp[:], Gm[:, d - 1, Cb, :], Dt[:, d - 1, :],
                        start=(d == 1), stop=(d == 7),
                    )
                nc.vector.tensor_copy(fmbf[0:121, Cb, :], fmp[:])
                for mt in range(8):
                    nc.tensor.matmul(
                        m1acc(mt), W1b[:, Cb, mt * 128:(mt + 1) * 128],
                        fmbf[:, Cb, :],
                        start=False, stop=False,
                    )

            # ---- mm1 x-half kt 4..7 + activations + W2 (offset by one mt so
            # ---- the W2 dot never waits on the just-issued activation) ----
            psf = pfr.tile([128, 2 * BC], F32, tag="pfr", name="psf")
            for mt in range(8):
                for kt in range(4, 8):
                    nc.tensor.matmul(
                        m1acc(mt), W1a[:, kt, mt * 128:(mt + 1) * 128],
                        x[:, kt, :],
                        start=False, stop=(kt == 7),
                    )
                nc.scalar.activation(
                    out=h[:, mt, :], in_=m1acc(mt), func=AF.Lrelu,
                    bias=bias[:, 8 + mt:9 + mt], scale=1.0, alpha=0.01,
                )
                if mt >= 1:
                    nc.tensor.matmul(
                        psf[0:1, 0:BC], W2[:, mt - 1, :], h[:, mt - 1, :],
                        start=(mt == 1), stop=False,
                    )
            nc.tensor.matmul(
                psf[0:1, 0:BC], W2[:, 7, :], h[:, 7, :],
                start=False, stop=True,
            )
            nc.scalar.activation(
                out=sig[:], in_=psf[0:1, 0:BC], func=AF.Sigmoid, bias=b2_val,
                scale=1.0,
            )
            nc.sync.dma_start(out_d[:], sig[:])
        _xpad_free()

    nc.finalize()
    return nc


def _prep_shared(inputs):
    """Host-side weight prep shared across cores."""
    bf16 = ml_dtypes.bfloat16
    fp8 = ml_dtypes.float8_e4m3fn
    Wd = np.asarray(inputs["W_d"], np.float32)
    bd = np.asarray(inputs["b_d"], np.float32)
    Wc = np.asarray(inputs["W_c"], np.float32)
    bc = np.asarray(inputs["b_c"], np.float32)
    v = np.asarray(inputs["v"], np.float32)[0]          # [CHANNEL, FIELD, EMB]
    lin_w = np.asarray(inputs["lin_w"], np.float32)     # [FIELD, 1]
    lin_b = np.asarray(inputs["lin_b"], np.float32)     # [1]
    W1 = np.asarray(inputs["W1"], np.float32)           # [2041, HID]
    b1 = np.asarray(inputs["b1"], np.float32)
    W2 = np.asarray(inputs["W2"], np.float32)           # [HID, 1]

    # banded FM weights, 121-channel blocking:
    # Gm[p, d-1, Cb, m] = G[c=121*Cb+m, f=p-m, f+d] for 0 <= p-m < 8-d
    G = np.einsum("cfe,cge->cfg", v, v)                 # [CHANNEL, 8, 8]
    Gm = np.zeros((128, 7, NCB, CB), np.float32)
    m_idx = np.arange(CB)
    for d in range(1, 8):
        for Cb in range(NCB):
            c = CB * Cb + m_idx
            for f in range(0, 8 - d):
                p = m_idx + f
                ok = c < CHANNEL
                Gm[p[ok], d - 1, Cb, m_idx[ok]] = G[c[ok], f, f + d]

    # fold the FM linear term (x_fm @ lin_w + lin_b) into W1's top half / b1
    W1a = W1[:NH0].copy()                               # [1024, HID]
    W1b = W1[NH0:]                                      # [CHANNEL, HID]
    for f in range(FIELD):
        W1a[f:f + CHANNEL, :] += lin_w[f, 0] * W1b
    b1e = b1 + lin_b[0] * W1b.sum(0)

    W1a_p = np.zeros((1024, 1024), np.float32)
    W1a_p[:, :HID] = W1a
    # W1b re-blocked by 121-channel windows: row p of block Cb = channel
    # 121*Cb + p (p < 121; p >= 121 stays zero)
    W1b_p = np.zeros((128, NCB, 1024), np.float32)
    for Cb in range(NCB):
        n = min(CB, CHANNEL - CB * Cb)
        W1b_p[:n, Cb, :HID] = W1b[CB * Cb:CB * Cb + n, :]
    b1_p = np.zeros(1024, np.float32)
    b1_p[:HID] = b1e
    W2_p = np.zeros(1024, np.float32)
    W2_p[:HID] = W2[:, 0]

    Wf = np.concatenate([
        Wd.reshape(4, 128, 512).transpose(1, 0, 2),
        Wc.reshape(2, 128, 512).transpose(1, 0, 2),
    ], axis=1)                                          # [128, 6, 512]
    bias_all = np.concatenate([
        bd.reshape(4, 128).T, bc.reshape(4, 128).T,
        np.ascontiguousarray(b1_p.reshape(8, 128).T),
    ], axis=1)                                          # [128, 16]

    shared = {
        "Wf": np.ascontiguousarray(Wf).astype(bf16),
        "Gm": Gm.astype(bf16),
        "W1a": np.ascontiguousarray(
            W1a_p.reshape(8, 128, 1024).transpose(1, 0, 2)).astype(bf16),
        "W1b": W1b_p.astype(bf16),
        "bias": np.ascontiguousarray(bias_all, dtype=np.float32),
        "W2": np.ascontiguousarray(
            W2_p.reshape(8, 128).T)[:, :, None].astype(bf16),
    }
    b2_val = float(np.asarray(inputs["b2"], np.float32)[0])
    return shared, b2_val


def build_in_maps(inputs):
    dx = np.asarray(inputs["discrete_x"], np.float32)   # [B, NDF]
    cx = np.asarray(inputs["continous_x"], np.float32)  # [B, NCF]
    shared, b2_val = _prep_shared(inputs)
    bf16 = ml_dtypes.bfloat16

    in_maps = []
    for i in range(NCORES):
        dxi = dx[i * BC:(i + 1) * BC]                   # [BC, NDF]
        cxi = cx[i * BC:(i + 1) * BC]
        m = dict(shared)
        m["xin"] = np.ascontiguousarray(np.concatenate([
            dxi.T.reshape(4, 128, BC).transpose(1, 0, 2),
            cxi.T.reshape(2, 128, BC).transpose(1, 0, 2),
        ], axis=1)).astype(bf16)
        in_maps.append(m)
    return in_maps, b2_val


def kernel(**inputs) -> np.ndarray:
    in_maps, b2_val = build_in_maps(inputs)

    key = "nc"
    if key not in _cache or _cache.get("b2") != b2_val:
        _cache[key] = _build(b2_val)
        _cache["b2"] = b2_val
    nc = _cache[key]

    res = run_bass_kernel_spmd(nc, in_maps, core_ids=list(range(NCORES)))
    out = np.empty((B, 1), np.float32)
    for i in range(NCORES):
        out[i * BC:(i + 1) * BC, 0] = res.results[i]["out"][0]
    return out


# revision 16
# speedup vs baseline: 1.0041x; 1.0041x over previous
"""Trainium2 Bass kernel for nn_FIN_b: windowed-FM tabular net.

Data-parallel over batch: B=2048 rows split across 8 NeuronCores (256 each).
Activations are feature-major ([feature_partition, batch_free]).  Front and
big matmuls run in bf16, the FM block in fp8-e4m3 (both fine vs the required
tolerance), fp32 PSUM accumulate everywhere.  The windowed FM
    fm_out[b,c] = sum_{d=1..7} sum_f D_d[b,c+f] G[c,f,f+d],
    D_d = x * shift_d(x),  G[c,f,g] = sum_e v[c,f,e] v[c,g,e]
runs in nine 121-channel blocks: x is written (fp8) to a feature-major DRAM
scratch as each front activation finishes; for block Cb ONE linear DMA
fetches rows 121*Cb + p + d (p=0..127, d=0..7) giving the block's x window
plus all 7 shifted windows (121+7 == 128, so no boundary stragglers), one
broadcast multiply forms the 7 D_d products, 7 banded matmuls accumulate fm,
and the block's 8 W1b contributions to the big matmul are issued right away.
The big-matmul x-half streams afterwards, interleaved with activations and
the W2 dot per output tile.  The FM linear term (x_fm @ lin_w) is folded
into W1's top half on the host.

Scheduling notes (why this is fast):
 - The PE p-state ramps to full clock only under continuous load, so the
   emission order keeps one long tensor stream: front (kt-outer) -> per-block
   FM+W1b -> W1a+W2.
 - The two HW DGE queues are split so small data-path packets never fight
   bulk weight packets for DMA-engine slots: Activation queue carries
   bias/Wf/Gm then the nine shifted reads; Sync queue carries xin, the x
   writes, then W1b (nine chunks) / W1a / out strictly FIFO.
"""

import sys

sys.path.insert(0, "/opt/trn_rl_repo")

import numpy as np
import ml_dtypes

import concourse.bass as bass
import concourse.tile as tile
from concourse import bacc, mybir
from concourse.bass_utils import run_bass_kernel_spmd

NDF, NCF, NCC = 512, 256, 256
EMB, FIELD = 16, 8
B = 2048
NH0 = NDF + 2 * NCC          # 1024
CHANNEL = NH0 - FIELD + 1    # 1017
HID = (NH0 + CHANNEL) // 2   # 1020
NCORES = 8
BC = B // NCORES             # 256 batch rows per core
CB = 121                     # channels per FM block (121 + 7 shifts = 128)
NCB = 9                      # ceil(CHANNEL / CB)
XPAD = CB * (NCB - 1) + 128 + FIELD   # pad rows so every block read is legal

F32 = mybir.dt.float32
BF16 = mybir.dt.bfloat16
FP8 = mybir.dt.bfloat16  # FM path dtype (DVE fp8 is 1.8x slower)

_cache = {}


def _build(b2_val: float):
    nc = bacc.Bacc()

    xin_d = nc.dram_tensor("xin", [128, 6, BC], BF16, kind="ExternalInput")
    Wf_d = nc.dram_tensor("Wf", [128, 6, 512], BF16, kind="ExternalInput")
    Gm_d = nc.dram_tensor("Gm", [128, 7, NCB, CB], FP8, kind="ExternalInput")
    W1a_d = nc.dram_tensor("W1a", [128, 8, 1024], BF16, kind="ExternalInput")
    W1b_d = nc.dram_tensor("W1b", [128, NCB, 1024], BF16, kind="ExternalInput")
    bias_d = nc.dram_tensor("bias", [128, 16], F32, kind="ExternalInput")
    W2_d = nc.dram_tensor("W2", [128, 8, 1], BF16, kind="ExternalInput")
    out_d = nc.dram_tensor("out", [1, BC], F32, kind="ExternalOutput")

    AF = mybir.ActivationFunctionType

    with tile.TileContext(nc) as tc:
        xpad, _xpad_free = tc.tile([XPAD, BC], FP8, space="DRAM", name="xpad")
        with (
            tc.tile_pool(name="w", bufs=1) as wp,
            tc.tile_pool(name="act", bufs=1) as ap,
            tc.tile_pool(name="xa", bufs=3) as xap,
            tc.tile_pool(name="dt", bufs=2) as dtp,
            tc.tile_pool(name="pfr", bufs=3, space=bass.MemorySpace.PSUM) as pfr,
            tc.tile_pool(name="pm1", bufs=1, space=bass.MemorySpace.PSUM) as pm1,
            tc.tile_pool(name="pfm", bufs=1, space=bass.MemorySpace.PSUM) as pfm,
        ):
            # ---- Activation-queue DMAs (consumption order: c-front chunk
            # ---- first, the front is computed c-part first) ----
            bias = wp.tile([128, 16], F32, tag="bias")
            nc.scalar.dma_start(bias[:], bias_d[:])
            Wf = wp.tile([128, 6, 512], BF16, tag="Wf")
            nc.scalar.dma_start(Wf[:, 4:6, :], Wf_d[:, 4:6, :])
            nc.scalar.dma_start(Wf[:, 0, :], Wf_d[:, 0, :])
            nc.scalar.dma_start(Wf[:, 1, :], Wf_d[:, 1, :])
            nc.scalar.dma_start(Wf[:, 2:4, :], Wf_d[:, 2:4, :])
            Gm = wp.tile([128, 7, NCB, CB], FP8, tag="Gm")
            nc.scalar.dma_start(Gm[:, :, 0:5, :], Gm_d[:, :, 0:5, :])
            nc.scalar.dma_start(Gm[:, :, 5:NCB, :], Gm_d[:, :, 5:NCB, :])
            W2 = wp.tile([128, 8, 1], BF16, tag="W2")
            nc.scalar.dma_start(W2[:], W2_d[:])

            # ---- Sync-queue: xin, then W1a / W1b stream uncontended ----
            xin = wp.tile([128, 6, BC], BF16, tag="xin")
            nc.sync.dma_start(xin[:], xin_d[:])
            W1a = wp.tile([128, 8, 1024], BF16, tag="W1a")
            for kt in range(8):
                nc.sync.dma_start(W1a[:, kt, :], W1a_d[:, kt, :])
            W1b = wp.tile([128, NCB, 1024], BF16, tag="W1b")
            for j in range(3):
                nc.sync.dma_start(W1b[:, 3 * j:3 * j + 3, :],
                                  W1b_d[:, 3 * j:3 * j + 3, :])

            zst = ap.tile([128, 128], BF16, tag="zst")
            nc.vector.memset(zst[:], 0.0)
            zer = ap.tile([XPAD - NH0, BC], FP8, tag="zer")
            nc.vector.memset(zer[:], 0.0)
            nc.scalar.dma_start(xpad[NH0:XPAD, :], zer[:])

            x = ap.tile([128, 8, BC], BF16, tag="x")
            fmbf = ap.tile([128, NCB, BC], BF16, tag="fmbf")
            nc.vector.memset(fmbf[:], 0.0)
            h = ap.tile([128, 8, BC], BF16, tag="h")
            sig = ap.tile([1, BC], F32, tag="sig")

            # ---- front ----
            # a matmul with start=True clears its whole PSUM bank, so paired
            # accumulator banks are zeroed once with a zero-stationary matmul
            # and every real chain accumulates with start=False.
            # c-part (features 512..1023) first: its Wf chunk arrives first
            # and its activations then overlap the d-part matmuls.
            cps = [pfr.tile([128, 2 * BC], F32, tag="pfr", name=f"cps{j}")
                   for j in range(2)]
            cacc = lambda mt: cps[mt // 2][:, (mt % 2) * BC:(mt % 2 + 1) * BC]
            for j in range(2):
                nc.tensor.matmul(cps[j][:], zst[:], Wf[:, 4, :],
                                 start=True, stop=False)
            for kt in range(2):
                for mt in range(4):
                    nc.tensor.matmul(
                        cacc(mt), Wf[:, 4 + kt, mt * 128:(mt + 1) * 128],
                        xin[:, 4 + kt, :], start=False, stop=(kt == 1),
                    )
            # mm1 accumulators zeroed while the d-part Wf chunks arrive
            m1ps = [pm1.tile([128, 2 * BC], F32, tag=f"pm1{j}", name=f"pm1{j}")
                    for j in range(4)]
            m1acc = lambda mt: m1ps[mt // 2][:, (mt % 2) * BC:(mt % 2 + 1) * BC]
            for j in range(4):
                nc.tensor.matmul(m1ps[j][:], zst[:], Wf[:, 4, :],
                                 start=True, stop=False)
            # d-part in two passes of two output tiles each
            dps0 = pfr.tile([128, 2 * BC], F32, tag="pfr", name="dps0")
            nc.tensor.matmul(dps0[:], zst[:], Wf[:, 4, :], start=True,
                             stop=False)
            for kt in range(4):
                for mt in range(2):
                    nc.tensor.matmul(
                        dps0[:, mt * BC:(mt + 1) * BC],
                        Wf[:, kt, mt * 128:(mt + 1) * 128],
                        xin[:, kt, :], start=False, stop=(kt == 3),
                    )
            # c-part activations + write (frees banks for the second d pass)
            for mt in range(4):
                nc.scalar.activation(
                    out=x[:, 4 + mt, :], in_=cacc(mt), func=AF.Lrelu,
                    bias=bias[:, 4 + mt:5 + mt], scale=1.0, alpha=0.0,
                )
            nc.scalar.dma_start(
                xpad[512:1024, :].rearrange("(g p) b -> p g b", p=128),
                x[:, 4:8, :])
            dps1 = pfr.tile([128, 2 * BC], F32, tag="pfr", name="dps1")
            nc.tensor.matmul(dps1[:], zst[:], Wf[:, 4, :], start=True,
                             stop=False)
            for kt in range(4):
                for mt in range(2):
                    nc.tensor.matmul(
                        dps1[:, mt * BC:(mt + 1) * BC],
                        Wf[:, kt, (2 + mt) * 128:(3 + mt) * 128],
                        xin[:, kt, :], start=False, stop=(kt == 3),
                    )
            # d-part activations + writes, then the first shifted read
            dacc = lambda mt: (dps0 if mt < 2 else dps1)[
                :, (mt % 2) * BC:(mt % 2 + 1) * BC]
            for mt in range(4):
                nc.scalar.activation(
                    out=x[:, mt, :], in_=dacc(mt), func=AF.Lrelu,
                    bias=bias[:, mt:mt + 1], scale=1.0, alpha=0.0,
                )
                if mt % 2 == 1:
                    nc.scalar.dma_start(
                        xpad[(mt - 1) * 128:(mt + 1) * 128, :]
                        .rearrange("(g p) b -> p g b", p=128),
                        x[:, mt - 1:mt + 1, :])

            def xa_read(Cb):
                xa = xap.tile([128, 8, BC], FP8, tag="xa", name="xa")
                # xa[p, d, :] = xpad[121*Cb + p + d, :]  (one linear DMA)
                src = bass.AP(xpad.tensor, CB * Cb * BC,
                              [[BC, 128], [BC, 8], [1, BC]])
                nc.scalar.dma_start(xa[:], src)
                return xa
            xas = [xa_read(0)]

            # x-half of the big matmul, kt 0..5: fills the tensor bubble while
            # the first shifted read + D product are in flight
            for kt in range(6):
                for mt in range(8):
                    nc.tensor.matmul(
                        m1acc(mt), W1a[:, kt, mt * 128:(mt + 1) * 128],
                        x[:, kt, :], start=False, stop=False,
                    )

            # ---- FM + W1b pipeline over nine 121-channel blocks ----
            for Cb in range(NCB):
                xa = xas[Cb]
                if Cb + 1 < NCB:
                    xas.append(xa_read(Cb + 1))
                Dt = dtp.tile([128, 7, BC], FP8, tag="Dt")
                nc.vector.tensor_mul(
                    Dt[:], xa[:, 0:1, :].broadcast_to([128, 7, BC]),
                    xa[:, 1:8, :])
                fmp = pfm.tile([121, BC], F32, tag="pfm", name="fmp")
                for d in range(1, 8):
                    nc.tensor.matmul(
                        fmp[:], Gm[:, d - 1, Cb, :], Dt[:, d - 1, :],
                        start=(d == 1), stop=(d == 7),
                    )
                nc.vector.tensor_copy(fmbf[0:121, Cb, :], fmp[:])
                for mt in range(8):
                    nc.tensor.matmul(
                        m1acc(mt), W1b[:, Cb, mt * 128:(mt + 1) * 128],
                        fmbf[:, Cb, :],
                        start=False, stop=False,
                    )

            # ---- mm1 x-half kt 6..7 + activations + W2 (offset by one mt so
            # ---- the W2 dot never waits on the just-issued activation) ----
            psf = pfr.tile([128, 2 * BC], F32, tag="pfr", name="psf")
            for mt in range(8):
                for kt in range(6, 8):
                    nc.tensor.matmul(
                        m1acc(mt), W1a[:, kt, mt * 128:(mt + 1) * 128],
                        x[:, kt, :],
                        start=False, stop=(kt == 7),
                    )
                nc.scalar.activation(
                    out=h[:, mt, :], in_=m1acc(mt), func=AF.Lrelu,
                    bias=bias[:, 8 + mt:9 + mt], scale=1.0, alpha=0.01,
                )
                if mt >= 1:
                    nc.tensor.matmul(
                        psf[0:1, 0:BC], W2[:, mt - 1, :], h[:, mt - 1, :],
                        start=(mt == 1), stop=False,
                    )
            nc.tensor.matmul(
                psf[0:1, 0:BC], W2[:, 7, :], h[:, 7, :],
                start=False, stop=True,
            )
            nc.scalar.activation(
                out=sig[:], in_=psf[0:1, 0:BC], func=AF.Sigmoid, bias=b2_val,
                scale=1.0,
            )
            nc.sync.dma_start(out_d[:], sig[:])
        _xpad_free()

    nc.finalize()
    return nc


def _prep_shared(inputs):
    """Host-side weight prep shared across cores."""
    bf16 = ml_dtypes.bfloat16
    fp8 = ml_dtypes.float8_e4m3fn
    Wd = np.asarray(inputs["W_d"], np.float32)
    bd = np.asarray(inputs["b_d"], np.float32)
    Wc = np.asarray(inputs["W_c"], np.float32)
    bc = np.asarray(inputs["b_c"], np.float32)
    v = np.asarray(inputs["v"], np.float32)[0]          # [CHANNEL, FIELD, EMB]
    lin_w = np.asarray(inputs["lin_w"], np.float32)     # [FIELD, 1]
    lin_b = np.asarray(inputs["lin_b"], np.float32)     # [1]
    W1 = np.asarray(inputs["W1"], np.float32)           # [2041, HID]
    b1 = np.asarray(inputs["b1"], np.float32)
    W2 = np.asarray(inputs["W2"], np.float32)           # [HID, 1]

    # banded FM weights, 121-channel blocking:
    # Gm[p, d-1, Cb, m] = G[c=121*Cb+m, f=p-m, f+d] for 0 <= p-m < 8-d
    G = np.einsum("cfe,cge->cfg", v, v)                 # [CHANNEL, 8, 8]
    Gm = np.zeros((128, 7, NCB, CB), np.float32)
    m_idx = np.arange(CB)
    for d in range(1, 8):
        for Cb in range(NCB):
            c = CB * Cb + m_idx
            for f in range(0, 8 - d):
                p = m_idx + f
                ok = c < CHANNEL
                Gm[p[ok], d - 1, Cb, m_idx[ok]] = G[c[ok], f, f + d]

    # fold the FM linear term (x_fm @ lin_w + lin_b) into W1's top half / b1
    W1a = W1[:NH0].copy()                               # [1024, HID]
    W1b = W1[NH0:]                                      # [CHANNEL, HID]
    for f in range(FIELD):
        W1a[f:f + CHANNEL, :] += lin_w[f, 0] * W1b
    b1e = b1 + lin_b[0] * W1b.sum(0)

    W1a_p = np.zeros((1024, 1024), np.float32)
    W1a_p[:, :HID] = W1a
    # W1b re-blocked by 121-channel windows: row p of block Cb = channel
    # 121*Cb + p (p < 121; p >= 121 stays zero)
    W1b_p = np.zeros((128, NCB, 1024), np.float32)
    for Cb in range(NCB):
        n = min(CB, CHANNEL - CB * Cb)
        W1b_p[:n, Cb, :HID] = W1b[CB * Cb:CB * Cb + n, :]
    b1_p = np.zeros(1024, np.float32)
    b1_p[:HID] = b1e
    W2_p = np.zeros(1024, np.float32)
    W2_p[:HID] = W2[:, 0]

    Wf = np.concatenate([
        Wd.reshape(4, 128, 512).transpose(1, 0, 2),
        Wc.reshape(2, 128, 512).transpose(1, 0, 2),
    ], axis=1)                                          # [128, 6, 512]
    bias_all = np.concatenate([
        bd.reshape(4, 128).T, bc.reshape(4, 128).T,
        np.ascontiguousarray(b1_p.reshape(8, 128).T),
    ], axis=1)                                          # [128, 16]

    shared = {
        "Wf": np.ascontiguousarray(Wf).astype(bf16),
        "Gm": Gm.astype(bf16),
        "W1a": np.ascontiguousarray(
            W1a_p.reshape(8, 128, 1024).transpose(1, 0, 2)).astype(bf16),
        "W1b": W1b_p.astype(bf16),
        "bias": np.ascontiguousarray(bias_all, dtype=np.float32),
        "W2": np.ascontiguousarray(
            W2_p.reshape(8, 128).T)[:, :, None].astype(bf16),
    }
    b2_val = float(np.asarray(inputs["b2"], np.float32)[0])
    return shared, b2_val


def build_in_maps(inputs):
    dx = np.asarray(inputs["discrete_x"], np.float32)   # [B, NDF]
    cx = np.asarray(inputs["continous_x"], np.float32)  # [B, NCF]
    shared, b2_val = _prep_shared(inputs)
    bf16 = ml_dtypes.bfloat16

    in_maps = []
    for i in range(NCORES):
        dxi = dx[i * BC:(i + 1) * BC]                   # [BC, NDF]
        cxi = cx[i * BC:(i + 1) * BC]
        m = dict(shared)
        m["xin"] = np.ascontiguousarray(np.concatenate([
            dxi.T.reshape(4, 128, BC).transpose(1, 0, 2),
            cxi.T.reshape(2, 128, BC).transpose(1, 0, 2),
        ], axis=1)).astype(bf16)
        in_maps.append(m)
    return in_maps, b2_val


def kernel(**inputs) -> np.ndarray:
    in_maps, b2_val = build_in_maps(inputs)

    key = "nc"
    if key not in _cache or _cache.get("b2") != b2_val:
        _cache[key] = _build(b2_val)
        _cache["b2"] = b2_val
    nc = _cache[key]

    res = run_bass_kernel_spmd(nc, in_maps, core_ids=list(range(NCORES)))
    out = np.empty((B, 1), np.float32)
    for i in range(NCORES):
        out[i * BC:(i + 1) * BC, 0] = res.results[i]["out"][0]
    return out


# revision 17
# speedup vs baseline: 1.0691x; 1.0647x over previous
"""Trainium2 Bass kernel for nn_FIN_b: windowed-FM tabular net.

Data-parallel over batch: B=2048 rows split across 8 NeuronCores (256 each).
Activations are feature-major ([feature_partition, batch_free]).  Front and
big matmuls run in bf16, the FM block in fp8-e4m3 (both fine vs the required
tolerance), fp32 PSUM accumulate everywhere.  The windowed FM
    fm_out[b,c] = sum_{d=1..7} sum_f D_d[b,c+f] G[c,f,f+d],
    D_d = x * shift_d(x),  G[c,f,g] = sum_e v[c,f,e] v[c,g,e]
runs in nine 121-channel blocks: x is written (fp8) to a feature-major DRAM
scratch as each front activation finishes; for block Cb ONE linear DMA
fetches rows 121*Cb + p + d (p=0..127, d=0..7) giving the block's x window
plus all 7 shifted windows (121+7 == 128, so no boundary stragglers), one
broadcast multiply forms the 7 D_d products, 7 banded matmuls accumulate fm,
and the block's 8 W1b contributions to the big matmul are issued right away.
The big-matmul x-half streams afterwards, interleaved with activations and
the W2 dot per output tile.  The FM linear term (x_fm @ lin_w) is folded
into W1's top half on the host.

Scheduling notes (why this is fast):
 - The PE p-state ramps to full clock only under continuous load, so the
   emission order keeps one long tensor stream: front (kt-outer) -> per-block
   FM+W1b -> W1a+W2.
 - The two HW DGE queues are split so small data-path packets never fight
   bulk weight packets for DMA-engine slots: Activation queue carries
   bias/Wf/Gm then the nine shifted reads; Sync queue carries xin, the x
   writes, then W1b (nine chunks) / W1a / out strictly FIFO.
"""

import sys

sys.path.insert(0, "/opt/trn_rl_repo")

import numpy as np
import ml_dtypes

import concourse.bass as bass
import concourse.tile as tile
from concourse import bacc, mybir
from concourse.bass_utils import run_bass_kernel_spmd

NDF, NCF, NCC = 512, 256, 256
EMB, FIELD = 16, 8
B = 2048
NH0 = NDF + 2 * NCC          # 1024
CHANNEL = NH0 - FIELD + 1    # 1017
HID = (NH0 + CHANNEL) // 2   # 1020
NCORES = 8
BC = B // NCORES             # 256 batch rows per core
CB = 121                     # channels per FM block (121 + 7 shifts = 128)
NCB = 9                      # ceil(CHANNEL / CB)
XPAD = CB * (NCB - 1) + 128 + FIELD   # pad rows so every block read is legal

F32 = mybir.dt.float32
BF16 = mybir.dt.bfloat16
FP8 = mybir.dt.bfloat16  # FM path dtype (DVE fp8 is 1.8x slower)

_cache = {}


def _build(b2_val: float):
    nc = bacc.Bacc()

    xin_d = nc.dram_tensor("xin", [128, 6, BC], BF16, kind="ExternalInput")
    Wf_d = nc.dram_tensor("Wf", [128, 6, 512], BF16, kind="ExternalInput")
    Gm_d = nc.dram_tensor("Gm", [128, 7, NCB, CB], FP8, kind="ExternalInput")
    W1a_d = nc.dram_tensor("W1a", [128, 8, 1024], BF16, kind="ExternalInput")
    W1b_d = nc.dram_tensor("W1b", [128, NCB, 1024], BF16, kind="ExternalInput")
    bias_d = nc.dram_tensor("bias", [128, 16], F32, kind="ExternalInput")
    W2_d = nc.dram_tensor("W2", [128, 8, 1], BF16, kind="ExternalInput")
    out_d = nc.dram_tensor("out", [1, BC], F32, kind="ExternalOutput")

    AF = mybir.ActivationFunctionType

    with tile.TileContext(nc) as tc:
        xpad, _xpad_free = tc.tile([XPAD, BC], FP8, space="DRAM", name="xpad")
        with (
            tc.tile_pool(name="w", bufs=1) as wp,
            tc.tile_pool(name="act", bufs=1) as ap,
            tc.tile_pool(name="xa", bufs=4) as xap,
            tc.tile_pool(name="dt", bufs=3) as dtp,
            tc.tile_pool(name="pfr", bufs=3, space=bass.MemorySpace.PSUM) as pfr,
            tc.tile_pool(name="pm1", bufs=1, space=bass.MemorySpace.PSUM) as pm1,
            tc.tile_pool(name="pfm", bufs=1, space=bass.MemorySpace.PSUM) as pfm,
        ):
            # ---- Activation-queue DMAs (consumption order: c-front chunk
            # ---- first, the front is computed c-part first) ----
            Wf = wp.tile([128, 6, 512], BF16, tag="Wf")
            nc.scalar.dma_start(Wf[:, 4:6, :], Wf_d[:, 4:6, :])
            nc.scalar.dma_start(Wf[:, 0, :], Wf_d[:, 0, :])
            nc.scalar.dma_start(Wf[:, 1, :], Wf_d[:, 1, :])
            nc.scalar.dma_start(Wf[:, 2:4, :], Wf_d[:, 2:4, :])
            Gm = wp.tile([128, 7, NCB, CB], FP8, tag="Gm")
            nc.scalar.dma_start(Gm[:, :, 0:5, :], Gm_d[:, :, 0:5, :])
            nc.scalar.dma_start(Gm[:, :, 5:NCB, :], Gm_d[:, :, 5:NCB, :])

            # ---- Sync-queue: xin/bias, then W1a / W1b stream uncontended ----
            xin = wp.tile([128, 6, BC], BF16, tag="xin")
            nc.sync.dma_start(xin[:], xin_d[:])
            bias = wp.tile([128, 16], F32, tag="bias")
            nc.sync.dma_start(bias[:], bias_d[:])
            W1a = wp.tile([128, 8, 1024], BF16, tag="W1a")
            for kt in range(8):
                nc.sync.dma_start(W1a[:, kt, :], W1a_d[:, kt, :])
            W1b = wp.tile([128, NCB, 1024], BF16, tag="W1b")
            for j in range(3):
                nc.sync.dma_start(W1b[:, 3 * j:3 * j + 3, :],
                                  W1b_d[:, 3 * j:3 * j + 3, :])
            W2 = wp.tile([128, 8, 1], BF16, tag="W2")
            nc.sync.dma_start(W2[:], W2_d[:])

            zst = ap.tile([128, 128], BF16, tag="zst")
            nc.vector.memset(zst[:], 0.0)
            zer = ap.tile([XPAD - NH0, BC], FP8, tag="zer")
            nc.vector.memset(zer[:], 0.0)
            nc.sync.dma_start(xpad[NH0:XPAD, :], zer[:])

            x = ap.tile([128, 8, BC], BF16, tag="x")
            fmbf = ap.tile([128, NCB, BC], BF16, tag="fmbf")
            nc.vector.memset(fmbf[:], 0.0)
            h = ap.tile([128, 8, BC], BF16, tag="h")
            sig = ap.tile([1, BC], F32, tag="sig")

            # ---- front ----
            # a matmul with start=True clears its whole PSUM bank, so paired
            # accumulator banks are zeroed once with a zero-stationary matmul
            # and every real chain accumulates with start=False.
            # c-part (features 512..1023) first: its Wf chunk arrives first
            # and its activations then overlap the d-part matmuls.
            cps = [pfr.tile([128, 2 * BC], F32, tag="pfr", name=f"cps{j}")
                   for j in range(2)]
            cacc = lambda mt: cps[mt // 2][:, (mt % 2) * BC:(mt % 2 + 1) * BC]
            for j in range(2):
                nc.tensor.matmul(cps[j][:], zst[:], Wf[:, 4, :],
                                 start=True, stop=False)
            for kt in range(2):
                for mt in range(4):
                    nc.tensor.matmul(
                        cacc(mt), Wf[:, 4 + kt, mt * 128:(mt + 1) * 128],
                        xin[:, 4 + kt, :], start=False, stop=(kt == 1),
                    )
            # mm1 accumulators zeroed while the d-part Wf chunks arrive
            m1ps = [pm1.tile([128, 2 * BC], F32, tag=f"pm1{j}", name=f"pm1{j}")
                    for j in range(4)]
            m1acc = lambda mt: m1ps[mt // 2][:, (mt % 2) * BC:(mt % 2 + 1) * BC]
            for j in range(4):
                nc.tensor.matmul(m1ps[j][:], zst[:], Wf[:, 4, :],
                                 start=True, stop=False)
            # d-part in two passes of two output tiles each
            dps0 = pfr.tile([128, 2 * BC], F32, tag="pfr", name="dps0")
            nc.tensor.matmul(dps0[:], zst[:], Wf[:, 4, :], start=True,
                             stop=False)
            for kt in range(4):
                for mt in range(2):
                    nc.tensor.matmul(
                        dps0[:, mt * BC:(mt + 1) * BC],
                        Wf[:, kt, mt * 128:(mt + 1) * 128],
                        xin[:, kt, :], start=False, stop=(kt == 3),
                    )
            # c-part activations + write (frees banks for the second d pass)
            for mt in range(4):
                nc.scalar.activation(
                    out=x[:, 4 + mt, :], in_=cacc(mt), func=AF.Lrelu,
                    bias=bias[:, 4 + mt:5 + mt], scale=1.0, alpha=0.0,
                )
            nc.scalar.dma_start(
                xpad[512:1024, :].rearrange("(g p) b -> p g b", p=128),
                x[:, 4:8, :])
            dps1 = pfr.tile([128, 2 * BC], F32, tag="pfr", name="dps1")
            nc.tensor.matmul(dps1[:], zst[:], Wf[:, 4, :], start=True,
                             stop=False)
            for kt in range(4):
                for mt in range(2):
                    nc.tensor.matmul(
                        dps1[:, mt * BC:(mt + 1) * BC],
                        Wf[:, kt, (2 + mt) * 128:(3 + mt) * 128],
                        xin[:, kt, :], start=False, stop=(kt == 3),
                    )
            # d-part activations + writes, then the first shifted read
            dacc = lambda mt: (dps0 if mt < 2 else dps1)[
                :, (mt % 2) * BC:(mt % 2 + 1) * BC]
            for mt in range(4):
                nc.scalar.activation(
                    out=x[:, mt, :], in_=dacc(mt), func=AF.Lrelu,
                    bias=bias[:, mt:mt + 1], scale=1.0, alpha=0.0,
                )
                if mt % 2 == 1:
                    nc.scalar.dma_start(
                        xpad[(mt - 1) * 128:(mt + 1) * 128, :]
                        .rearrange("(g p) b -> p g b", p=128),
                        x[:, mt - 1:mt + 1, :])

            def xa_read(Cb):
                xa = xap.tile([128, 8, BC], FP8, tag="xa", name="xa")
                # xa[p, d, :] = xpad[121*Cb + p + d, :]  (one linear DMA)
                src = bass.AP(xpad.tensor, CB * Cb * BC,
                              [[BC, 128], [BC, 8], [1, BC]])
                nc.scalar.dma_start(xa[:], src)
                return xa
            xas = [xa_read(0)]

            # x-half of the big matmul, kt 0..5: fills the tensor bubble while
            # the first shifted read + D product are in flight
            for kt in range(6):
                for mt in range(8):
                    nc.tensor.matmul(
                        m1acc(mt), W1a[:, kt, mt * 128:(mt + 1) * 128],
                        x[:, kt, :], start=False, stop=False,
                    )

            # ---- FM + W1b pipeline over nine 121-channel blocks ----
            for Cb in range(NCB):
                xa = xas[Cb]
                if Cb + 1 < NCB:
                    xas.append(xa_read(Cb + 1))
                Dt = dtp.tile([128, 7, BC], FP8, tag="Dt")
                nc.vector.tensor_mul(
                    Dt[:], xa[:, 0:1, :].broadcast_to([128, 7, BC]),
                    xa[:, 1:8, :])
                fmp = pfm.tile([121, BC], F32, tag="pfm", name="fmp")
                for d in range(1, 8):
                    nc.tensor.matmul(
                        fmp[:], Gm[:, d - 1, Cb, :], Dt[:, d - 1, :],
                        start=(d == 1), stop=(d == 7),
                    )
                nc.vector.tensor_copy(fmbf[0:121, Cb, :], fmp[:])
                for mt in range(8):
                    nc.tensor.matmul(
                        m1acc(mt), W1b[:, Cb, mt * 128:(mt + 1) * 128],
                        fmbf[:, Cb, :],
                        start=False, stop=False,
                    )

            # ---- mm1 x-half kt 6..7 + activations + W2 (offset by one mt so
            # ---- the W2 dot never waits on the just-issued activation) ----
            psf = pfr.tile([128, 2 * BC], F32, tag="pfr", name="psf")
            for mt in range(8):
                for kt in range(6, 8):
                    nc.tensor.matmul(
                        m1acc(mt), W1a[:, kt, mt * 128:(mt + 1) * 128],
                        x[:, kt, :],
                        start=False, stop=(kt == 7),
                    )
                nc.scalar.activation(
                    out=h[:, mt, :], in_=m1acc(mt), func=AF.Lrelu,
                    bias=bias[:, 8 + mt:9 + mt], scale=1.0, alpha=0.01,
                )
                if mt >= 1:
                    nc.tensor.matmul(
                        psf[0:1, 0:BC], W2[:, mt - 1, :], h[:, mt - 1, :],
                        start=(mt == 1), stop=False,
                    )
            nc.tensor.matmul(
                psf[0:1, 0:BC], W2[:, 7, :], h[:, 7, :],
                start=False, stop=True,
            )
            nc.scalar.activation(
                out=sig[:], in_=psf[0:1, 0:BC], func=AF.Sigmoid, bias=b2_val,
                scale=1.0,
            )
            nc.sync.dma_start(out_d[:], sig[:])
        _xpad_free()

    nc.finalize()
    return nc


def _prep_shared(inputs):
    """Host-side weight prep shared across cores."""
    bf16 = ml_dtypes.bfloat16
    fp8 = ml_dtypes.float8_e4m3fn
    Wd = np.asarray(inputs["W_d"], np.float32)
    bd = np.asarray(inputs["b_d"], np.float32)
    Wc = np.asarray(inputs["W_c"], np.float32)
    bc = np.asarray(inputs["b_c"], np.float32)
    v = np.asarray(inputs["v"], np.float32)[0]          # [CHANNEL, FIELD, EMB]
    lin_w = np.asarray(inputs["lin_w"], np.float32)     # [FIELD, 1]
    lin_b = np.asarray(inputs["lin_b"], np.float32)     # [1]
    W1 = np.asarray(inputs["W1"], np.float32)           # [2041, HID]
    b1 = np.asarray(inputs["b1"], np.float32)
    W2 = np.asarray(inputs["W2"], np.float32)           # [HID, 1]

    # banded FM weights, 121-channel blocking:
    # Gm[p, d-1, Cb, m] = G[c=121*Cb+m, f=p-m, f+d] for 0 <= p-m < 8-d
    G = np.einsum("cfe,cge->cfg", v, v)                 # [CHANNEL, 8, 8]
    Gm = np.zeros((128, 7, NCB, CB), np.float32)
    m_idx = np.arange(CB)
    for d in range(1, 8):
        for Cb in range(NCB):
            c = CB * Cb + m_idx
            for f in range(0, 8 - d):
                p = m_idx + f
                ok = c < CHANNEL
                Gm[p[ok], d - 1, Cb, m_idx[ok]] = G[c[ok], f, f + d]

    # fold the FM linear term (x_fm @ lin_w + lin_b) into W1's top half / b1
    W1a = W1[:NH0].copy()                               # [1024, HID]
    W1b = W1[NH0:]                                      # [CHANNEL, HID]
    for f in range(FIELD):
        W1a[f:f + CHANNEL, :] += lin_w[f, 0] * W1b
    b1e = b1 + lin_b[0] * W1b.sum(0)

    W1a_p = np.zeros((1024, 1024), np.float32)
    W1a_p[:, :HID] = W1a
    # W1b re-blocked by 121-channel windows: row p of block Cb = channel
    # 121*Cb + p (p < 121; p >= 121 stays zero)
    W1b_p = np.zeros((128, NCB, 1024), np.float32)
    for Cb in range(NCB):
        n = min(CB, CHANNEL - CB * Cb)
        W1b_p[:n, Cb, :HID] = W1b[CB * Cb:CB * Cb + n, :]
    b1_p = np.zeros(1024, np.float32)
    b1_p[:HID] = b1e
    W2_p = np.zeros(1024, np.float32)
    W2_p[:HID] = W2[:, 0]

    Wf = np.concatenate([
        Wd.reshape(4, 128, 512).transpose(1, 0, 2),
        Wc.reshape(2, 128, 512).transpose(1, 0, 2),
    ], axis=1)                                          # [128, 6, 512]
    bias_all = np.concatenate([
        bd.reshape(4, 128).T, bc.reshape(4, 128).T,
        np.ascontiguousarray(b1_p.reshape(8, 128).T),
    ], axis=1)                                          # [128, 16]

    shared = {
        "Wf": np.ascontiguousarray(Wf).astype(bf16),
        "Gm": Gm.astype(bf16),
        "W1a": np.ascontiguousarray(
            W1a_p.reshape(8, 128, 1024).transpose(1, 0, 2)).astype(bf16),
        "W1b": W1b_p.astype(bf16),
        "bias": np.ascontiguousarray(bias_all, dtype=np.float32),
        "W2": np.ascontiguousarray(
            W2_p.reshape(8, 128).T)[:, :, None].astype(bf16),
    }
    b2_val = float(np.asarray(inputs["b2"], np.float32)[0])
    return shared, b2_val


def build_in_maps(inputs):
    dx = np.asarray(inputs["discrete_x"], np.float32)   # [B, NDF]
    cx = np.asarray(inputs["continous_x"], np.float32)  # [B, NCF]
    shared, b2_val = _prep_shared(inputs)
    bf16 = ml_dtypes.bfloat16

    in_maps = []
    for i in range(NCORES):
        dxi = dx[i * BC:(i + 1) * BC]                   # [BC, NDF]
        cxi = cx[i * BC:(i + 1) * BC]
        m = dict(shared)
        m["xin"] = np.ascontiguousarray(np.concatenate([
            dxi.T.reshape(4, 128, BC).transpose(1, 0, 2),
            cxi.T.reshape(2, 128, BC).transpose(1, 0, 2),
        ], axis=1)).astype(bf16)
        in_maps.append(m)
    return in_maps, b2_val


def kernel(**inputs) -> np.ndarray:
    in_maps, b2_val = build_in_maps(inputs)

    key = "nc"
    if key not in _cache or _cache.get("b2") != b2_val:
        _cache[key] = _build(b2_val)
        _cache["b2"] = b2_val
    nc = _cache[key]

    res = run_bass_kernel_spmd(nc, in_maps, core_ids=list(range(NCORES)))
    out = np.empty((B, 1), np.float32)
    for i in range(NCORES):
        out[i * BC:(i + 1) * BC, 0] = res.results[i]["out"][0]
    return out
